# revision 1
# baseline (speedup 1.0000x reference)
"""LIF layer (T=64, B=128, 2048->2048) on 8 trn2 NeuronCores.

Strategy: tensor-parallel over out_dim (each core owns 256 output
channels, sees the full x_seq). Per core:
  GEMM  cur[o, t*B+b] = sum_i W[o,i] * x[t,b,i]   (W stationary in PE)
  SCAN  64 sequential LIF steps on [128, 2, 128] tiles (DVE), reading
        cur straight out of PSUM.
Bias is folded away via the change of variable u = mem - b/(1-decay),
turning the per-step bias add into a per-channel spike threshold.

Host-side prep (not on HW): transpose x to [I, T*B], slice/transpose W,
precompute threshold/init tiles, final output is a cheap transpose+concat.
"""

import math
import os

import numpy as np

import concourse.bacc as bacc
import concourse.bass as bass
import concourse.mybir as mybir
import concourse.tile as tile
from concourse import bass_utils

# Problem constants (hardcoded per contract)
T, B, I, O = 64, 128, 2048, 2048
N_CORES = 8
OL = O // N_CORES          # 256 out-channels per core
TB = T * B                 # 8192 rows
KT = I // 128              # 16 k-tiles
NPB_COLS = 1024            # tb-columns per block (= 8 timesteps)
N_NPB = TB // NPB_COLS     # 8 blocks
MM_N = 512                 # moving free dim per matmul (psum bank)
TAU, THR = 2.0, 1.0
DECAY = math.exp(-1.0 / TAU)

F32 = mybir.dt.float32
ALU = mybir.AluOpType

# GEMM precision mode: "fp32" (exact, 4 cyc/row), "f32r" (fp22 inputs,
# 1 cyc/row), "bf16x3" (3-pass hi/lo split, ~exact, 3 cyc/row)
MODE = os.environ.get("LIF_MODE", "bf16x3")

_cache = {}


def _build_nc(mode):
    nc = bacc.Bacc(trn_type="TRN2", target_bir_lowering=False)

    mm_dt = {"fp32": F32, "f32r": mybir.dt.float32r, "bf16x3": mybir.dt.bfloat16}[mode]

    # DRAM I/O. "stages": list of (x_dram, [w_variant_idx,...]) — each stage
    # loads its x tile once and runs matmuls against each listed w variant,
    # all accumulating into the same PSUM group.
    if mode == "bf16x3":
        xT_h = nc.dram_tensor("xT_h", [I, TB], mybir.dt.bfloat16, kind="ExternalInput")
        xT_l = nc.dram_tensor("xT_l", [I, TB], mybir.dt.bfloat16, kind="ExternalInput")
        n_wv = 2
        stages = [(xT_h, [0, 1]), (xT_l, [0])]   # xh@wh + xh@wl + xl@wh
    else:
        xT = nc.dram_tensor("xT", [I, TB], F32, kind="ExternalInput")
        n_wv = 1
        stages = [(xT, [0])]
    # weights pre-packed on host to w_all's exact SBUF layout -> one fast DMA
    w_packed = nc.dram_tensor("w_packed", [128, n_wv, KT, 2, 128], mm_dt,
                              kind="ExternalInput")
    n_mm_grp = sum(len(wvs) for _, wvs in stages) * KT  # accum group length
    thr_d = nc.dram_tensor("thr", [128, 2, 128], F32, kind="ExternalInput")
    u0_d = nc.dram_tensor("u0", [128, 2, 128], F32, kind="ExternalInput")
    out_d = nc.dram_tensor("out", [128, 2, T, B], F32, kind="ExternalOutput")

    with tile.TileContext(nc) as tc:
        with (
            tc.tile_pool(name="wpool", bufs=1) as wpool,
            tc.tile_pool(name="xpool", bufs=8) as xpool,
            tc.tile_pool(name="state", bufs=1) as state,
            tc.tile_pool(name="spkpool", bufs=4) as spkpool,
            tc.tile_pool(name="psum", bufs=8, space="PSUM") as psum_pool,
        ):
            # Preload weight tiles, one contiguous DMA per variant (gpsimd
            # queue, parallel to the x-prefetch on the sync queue); first
            # matmul only gates on variant 0.
            w_all = wpool.tile([128, n_wv, KT, 2, 128], mm_dt)
            for wv in range(n_wv):
                nc.gpsimd.dma_start(w_all[:, wv], w_packed[:, wv])

            # Persistent state tiles
            u = state.tile([128, 2, 128], F32)
            thr_t = state.tile([128, 2, 128], F32)
            nc.gpsimd.dma_start(u[:], u0_d[:])
            nc.gpsimd.dma_start(thr_t[:], thr_d[:])

            # col-blocks: 1024-wide except the last 1024 split in two, so the
            # final exposed scan (after the last matmul) is only 4 steps
            blocks = [(i * NPB_COLS, NPB_COLS) for i in range(N_NPB - 1)]
            blocks += [(TB - 1024, 512), (TB - 512, 512)]
            for bi, (cs, ncols) in enumerate(blocks):
                n_nn = ncols // MM_N
                # one psum tile per (ot, nn): [128, 512] fp32 = 1 bank
                ps = [[psum_pool.tile([128, MM_N], F32, tag="ps", name=f"ps_{bi}_{ot}_{nn}")
                       for nn in range(n_nn)] for ot in range(2)]
                mm_i = 0
                for x_src, wvs in stages:
                    for k in range(KT):
                        xt = xpool.tile([128, NPB_COLS], mm_dt, tag="xt",
                                        name=f"xt_{bi}_{k}")
                        nc.sync.dma_start(
                            xt[:, :ncols],
                            x_src[k * 128:(k + 1) * 128, cs:cs + ncols],
                        )
                        for wv in wvs:
                            mm_i += 1
                            for ot in range(2):
                                for nn in range(n_nn):
                                    nc.tensor.matmul(
                                        ps[ot][nn][:],
                                        w_all[:, wv, k, ot, :],
                                        xt[:, nn * MM_N:(nn + 1) * MM_N],
                                        start=(mm_i == 1),
                                        stop=(mm_i == n_mm_grp),
                                    )

                # LIF steps consuming this block's PSUM
                for tl in range(ncols // 128):
                    t = (cs // 128) + tl  # global timestep
                    nn, off = tl // 4, (tl % 4) * 128
                    nc.vector.tensor_scalar_mul(u[:], u[:], DECAY)
                    nc.vector.tensor_tensor(
                        u[:, 0, :], u[:, 0, :], ps[0][nn][:, off:off + 128], op=ALU.add)
                    nc.vector.tensor_tensor(
                        u[:, 1, :], u[:, 1, :], ps[1][nn][:, off:off + 128], op=ALU.add)
                    spk = spkpool.tile([128, 2, 128], F32, tag="spk")
                    nc.vector.tensor_tensor(spk[:], u[:], thr_t[:], op=ALU.is_gt)
                    nc.vector.tensor_tensor(u[:], u[:], spk[:], op=ALU.subtract)
                    nc.gpsimd.dma_start(out_d[:, :, t, :], spk[:])

    nc.compile()
    return nc


def _get_nc(mode):
    if mode not in _cache:
        _cache[mode] = _build_nc(mode)
    return _cache[mode]


def kernel(x_seq: np.ndarray, W: np.ndarray, b: np.ndarray) -> np.ndarray:
    mode = MODE
    nc = _get_nc(mode)

    x = np.ascontiguousarray(x_seq.reshape(TB, I), dtype=np.float32)
    xT = np.ascontiguousarray(x.T)  # [I, TB]

    if mode == "bf16x3":
        import ml_dtypes
        xT_h = xT.astype(ml_dtypes.bfloat16)
        xT_l = (xT - xT_h.astype(np.float32)).astype(ml_dtypes.bfloat16)

    in_maps = []
    for c in range(N_CORES):
        w_c = W[c * OL:(c + 1) * OL, :].astype(np.float32)      # [OL, I]
        wTc = np.ascontiguousarray(w_c.T)                       # [I, OL]
        b_c = b[c * OL:(c + 1) * OL].astype(np.float32)         # [OL]
        shift = b_c / (1.0 - DECAY)
        thr = (THR - shift).reshape(2, 128).transpose(1, 0)     # [128(op), 2(ot)]
        u0 = (-shift).reshape(2, 128).transpose(1, 0)
        thr_tile = np.ascontiguousarray(
            np.broadcast_to(thr[:, :, None], (128, 2, 128)), dtype=np.float32)
        u0_tile = np.ascontiguousarray(
            np.broadcast_to(u0[:, :, None], (128, 2, 128)), dtype=np.float32)
        m = {"thr": thr_tile, "u0": u0_tile}

        def pack_w(wt):  # [I, OL] -> [128(p), KT, 2(ot), 128(f)]
            return wt.reshape(KT, 128, 2, 128).transpose(1, 0, 2, 3)

        if mode == "bf16x3":
            wTc_h = wTc.astype(ml_dtypes.bfloat16)
            wTc_l = (wTc - wTc_h.astype(np.float32)).astype(ml_dtypes.bfloat16)
            wp = np.ascontiguousarray(
                np.stack([pack_w(wTc_h), pack_w(wTc_l)], axis=1))
            m.update(xT_h=xT_h, xT_l=xT_l, w_packed=wp)
        else:
            wp = np.ascontiguousarray(pack_w(wTc)[:, None])
            m.update(xT=xT, w_packed=wp)
        in_maps.append(m)

    res = bass_utils.run_bass_kernel_spmd(nc, in_maps, core_ids=list(range(N_CORES)))
    global LAST_RESULT
    LAST_RESULT = res

    # Assemble: out_c[op, ot, t, b] -> [t, b, ot*128+op]; concat over cores
    parts = []
    for c in range(N_CORES):
        oc = res.results[c]["out"]  # [128, 2, T, B]
        parts.append(oc.transpose(2, 3, 1, 0).reshape(T, B, 2 * 128))
    return np.ascontiguousarray(np.concatenate(parts, axis=2))


LAST_RESULT = None



# revision 2
# speedup vs baseline: 1.4422x; 1.4422x over previous
"""LIF layer (T=64, B=128, 2048->2048) on 8 trn2 NeuronCores.

Strategy: tensor-parallel over out_dim (each core owns 256 output
channels, sees the full x_seq). Per core:
  GEMM  cur[o, t*B+b] = sum_i W[o,i] * x[t,b,i]   (W stationary in PE)
  SCAN  64 sequential LIF steps on [128, 2, 128] tiles (DVE), reading
        cur straight out of PSUM.
Bias is folded away via the change of variable u = mem - b/(1-decay),
turning the per-step bias add into a per-channel spike threshold.

Host-side prep (not on HW): transpose x to [I, T*B], slice/transpose W,
precompute threshold/init tiles, final output is a cheap transpose+concat.
"""

import math
import os

import numpy as np

import concourse.bacc as bacc
import concourse.bass as bass
import concourse.mybir as mybir
import concourse.tile as tile
from concourse import bass_utils

# Problem constants (hardcoded per contract)
T, B, I, O = 64, 128, 2048, 2048
N_CORES = 8
OL = O // N_CORES          # 256 out-channels per core
TB = T * B                 # 8192 rows
KT = I // 128              # 16 k-tiles
NPB_COLS = 1024            # tb-columns per block (= 8 timesteps)
N_NPB = TB // NPB_COLS     # 8 blocks
MM_N = 512                 # moving free dim per matmul (psum bank)
TAU, THR = 2.0, 1.0
DECAY = math.exp(-1.0 / TAU)

F32 = mybir.dt.float32
ALU = mybir.AluOpType

# GEMM precision mode: "fp32" (exact, 4 cyc/row), "f32r" (fp22 inputs,
# 1 cyc/row), "bf16x3" (3-pass hi/lo split, ~exact, 3 cyc/row)
MODE = os.environ.get("LIF_MODE", "bf16x3")

_cache = {}


def _build_nc(mode):
    nc = bacc.Bacc(trn_type="TRN2", target_bir_lowering=False)

    mm_dt = {"fp32": F32, "f32r": mybir.dt.float32r, "bf16x3": mybir.dt.bfloat16}[mode]

    # DRAM I/O. "stages": list of (x_dram, [w_variant_idx,...]) — each stage
    # loads its x tile once and runs matmuls against each listed w variant,
    # all accumulating into the same PSUM group.
    if mode == "bf16x3":
        xT_h = nc.dram_tensor("xT_h", [I, TB], mybir.dt.bfloat16, kind="ExternalInput")
        xT_l = nc.dram_tensor("xT_l", [I, TB], mybir.dt.bfloat16, kind="ExternalInput")
        n_wv = 2
        stages = [(xT_h, [0, 1]), (xT_l, [0])]   # xh@wh + xh@wl + xl@wh
    else:
        xT = nc.dram_tensor("xT", [I, TB], mm_dt, kind="ExternalInput")
        n_wv = 1
        stages = [(xT, [0])]
    # weights pre-packed on host to w_all's exact SBUF layout -> one fast DMA
    w_packed = nc.dram_tensor("w_packed", [128, n_wv, KT, 2, 128], mm_dt,
                              kind="ExternalInput")
    n_mm_grp = sum(len(wvs) for _, wvs in stages) * KT  # accum group length
    thr_d = nc.dram_tensor("thr", [128, 2, 128], F32, kind="ExternalInput")
    u0_d = nc.dram_tensor("u0", [128, 2, 128], F32, kind="ExternalInput")
    out_d = nc.dram_tensor("out", [128, 2, T, B], F32, kind="ExternalOutput")

    with tile.TileContext(nc) as tc:
        with (
            tc.tile_pool(name="wpool", bufs=1) as wpool,
            tc.tile_pool(name="xpool", bufs=8) as xpool,
            tc.tile_pool(name="state", bufs=1) as state,
            tc.tile_pool(name="spkpool", bufs=4) as spkpool,
            tc.tile_pool(name="psum", bufs=8, space="PSUM") as psum_pool,
        ):
            # Preload weight tiles, one contiguous DMA per variant (gpsimd
            # queue, parallel to the x-prefetch on the sync queue); first
            # matmul only gates on variant 0.
            w_all = wpool.tile([128, n_wv, KT, 2, 128], mm_dt)
            for wv in range(n_wv):
                nc.gpsimd.dma_start(w_all[:, wv], w_packed[:, wv])

            # Persistent state tiles
            u = state.tile([128, 2, 128], F32)
            thr_t = state.tile([128, 2, 128], F32)
            nc.gpsimd.dma_start(u[:], u0_d[:])
            nc.gpsimd.dma_start(thr_t[:], thr_d[:])

            # col-blocks: 1024-wide except the last 1024 split in two, so the
            # final exposed scan (after the last matmul) is only 4 steps
            blocks = [(i * NPB_COLS, NPB_COLS) for i in range(N_NPB - 1)]
            blocks += [(TB - 1024, 512), (TB - 512, 512)]
            for bi, (cs, ncols) in enumerate(blocks):
                n_nn = ncols // MM_N
                # one psum tile per (ot, nn): [128, 512] fp32 = 1 bank
                ps = [[psum_pool.tile([128, MM_N], F32, tag="ps", name=f"ps_{bi}_{ot}_{nn}")
                       for nn in range(n_nn)] for ot in range(2)]
                mm_i = 0
                for x_src, wvs in stages:
                    for k in range(KT):
                        xt = xpool.tile([128, NPB_COLS], mm_dt, tag="xt",
                                        name=f"xt_{bi}_{k}")
                        nc.sync.dma_start(
                            xt[:, :ncols],
                            x_src[k * 128:(k + 1) * 128, cs:cs + ncols],
                        )
                        for wv in wvs:
                            mm_i += 1
                            for ot in range(2):
                                for nn in range(n_nn):
                                    nc.tensor.matmul(
                                        ps[ot][nn][:],
                                        w_all[:, wv, k, ot, :],
                                        xt[:, nn * MM_N:(nn + 1) * MM_N],
                                        start=(mm_i == 1),
                                        stop=(mm_i == n_mm_grp),
                                    )

                # LIF steps consuming this block's PSUM
                for tl in range(ncols // 128):
                    t = (cs // 128) + tl  # global timestep
                    nn, off = tl // 4, (tl % 4) * 128
                    nc.vector.tensor_scalar_mul(u[:], u[:], DECAY)
                    nc.vector.tensor_tensor(
                        u[:, 0, :], u[:, 0, :], ps[0][nn][:, off:off + 128], op=ALU.add)
                    nc.vector.tensor_tensor(
                        u[:, 1, :], u[:, 1, :], ps[1][nn][:, off:off + 128], op=ALU.add)
                    spk = spkpool.tile([128, 2, 128], F32, tag="spk")
                    nc.vector.tensor_tensor(spk[:], u[:], thr_t[:], op=ALU.is_gt)
                    nc.vector.tensor_tensor(u[:], u[:], spk[:], op=ALU.subtract)
                    nc.gpsimd.dma_start(out_d[:, :, t, :], spk[:])

    nc.compile()
    return nc


def _get_nc(mode):
    if mode not in _cache:
        _cache[mode] = _build_nc(mode)
    return _cache[mode]


def kernel(x_seq: np.ndarray, W: np.ndarray, b: np.ndarray) -> np.ndarray:
    mode = MODE
    nc = _get_nc(mode)

    x = np.ascontiguousarray(x_seq.reshape(TB, I), dtype=np.float32)
    xT = np.ascontiguousarray(x.T)  # [I, TB]

    if mode == "bf16x3":
        import ml_dtypes
        xT_h = xT.astype(ml_dtypes.bfloat16)
        xT_l = (xT - xT_h.astype(np.float32)).astype(ml_dtypes.bfloat16)

    in_maps = []
    for c in range(N_CORES):
        w_c = W[c * OL:(c + 1) * OL, :].astype(np.float32)      # [OL, I]
        wTc = np.ascontiguousarray(w_c.T)                       # [I, OL]
        b_c = b[c * OL:(c + 1) * OL].astype(np.float32)         # [OL]
        shift = b_c / (1.0 - DECAY)
        thr = (THR - shift).reshape(2, 128).transpose(1, 0)     # [128(op), 2(ot)]
        u0 = (-shift).reshape(2, 128).transpose(1, 0)
        thr_tile = np.ascontiguousarray(
            np.broadcast_to(thr[:, :, None], (128, 2, 128)), dtype=np.float32)
        u0_tile = np.ascontiguousarray(
            np.broadcast_to(u0[:, :, None], (128, 2, 128)), dtype=np.float32)
        m = {"thr": thr_tile, "u0": u0_tile}

        def pack_w(wt):  # [I, OL] -> [128(p), KT, 2(ot), 128(f)]
            return wt.reshape(KT, 128, 2, 128).transpose(1, 0, 2, 3)

        if mode == "bf16x3":
            wTc_h = wTc.astype(ml_dtypes.bfloat16)
            wTc_l = (wTc - wTc_h.astype(np.float32)).astype(ml_dtypes.bfloat16)
            wp = np.ascontiguousarray(
                np.stack([pack_w(wTc_h), pack_w(wTc_l)], axis=1))
            m.update(xT_h=xT_h, xT_l=xT_l, w_packed=wp)
        else:
            wp = np.ascontiguousarray(pack_w(wTc)[:, None])
            m.update(xT=xT, w_packed=wp)
        in_maps.append(m)

    res = bass_utils.run_bass_kernel_spmd(nc, in_maps, core_ids=list(range(N_CORES)))
    global LAST_RESULT
    LAST_RESULT = res

    # Assemble: out_c[op, ot, t, b] -> [t, b, ot*128+op]; concat over cores
    parts = []
    for c in range(N_CORES):
        oc = res.results[c]["out"]  # [128, 2, T, B]
        parts.append(oc.transpose(2, 3, 1, 0).reshape(T, B, 2 * 128))
    return np.ascontiguousarray(np.concatenate(parts, axis=2))


LAST_RESULT = None



# revision 4
# speedup vs baseline: 2.0056x; 1.3906x over previous
"""LIF layer (T=64, B=128, 2048->2048) on 8 trn2 NeuronCores.

Sharding: 4-way over out_dim x 2-way over batch. Core (g, h) owns
out channels [g*512, (g+1)*512) and batch rows [h*64, (h+1)*64).

Per core:
  GEMM  cur[o, (t,b)] = sum_i W[o,i] * x[t,b,i] as a single f32r pass
        (fp22-ish precision, 1 cyc/row, measured ~0.015 rel on spikes)
  SCAN  64 sequential LIF steps on [128, 4, 64] state tiles (DVE),
        reading cur straight out of PSUM.
Bias is folded away via the change of variable u = mem - b/(1-decay),
turning the per-step bias add into a per-channel spike threshold
(THR=1 so the reset subtract is just u -= spk).

Blocks of 512 columns (8 timesteps): PSUM tile [128, 4ot, 512] = 4
banks, double-buffered. Matmuls run ot-outer/kt-inner so each bank
sees a 16-matmul accumulation run (avoids bank-cycling micro-idles).

Host-side prep: transpose x to [I, T*64] per batch half, slice/pack W,
precompute threshold/init tiles; final output is a transpose+concat.
"""

import math

import numpy as np

import concourse.bacc as bacc
import concourse.mybir as mybir
import concourse.tile as tile
from concourse import bass_utils

# Problem constants (hardcoded per contract)
T, B, I, O = 64, 128, 2048, 2048
N_CORES = 8
GO, GB = 4, 2              # out-groups x batch-groups
OL = O // GO               # 512 out-channels per core
OT = OL // 128             # 4 out tiles
BL = B // GB               # 64 batch rows per core
COLS = T * BL              # 4096 (t,b) columns per core
KT = I // 128              # 16 k-tiles
NBLK = 8                   # col-blocks per core
BLK = COLS // NBLK         # 512 cols = 8 timesteps per block
TBLK = BLK // BL           # 8 timesteps per block
TAU, THR = 2.0, 1.0
DECAY = math.exp(-1.0 / TAU)

F32 = mybir.dt.float32
F32R = mybir.dt.float32r
ALU = mybir.AluOpType

MODE = "f32r-tp4dp2"

_cache = {}


def _build_nc():
    nc = bacc.Bacc(trn_type="TRN2", target_bir_lowering=False)

    xT_d = nc.dram_tensor("xT", [I, COLS], F32R, kind="ExternalInput")
    w_d = nc.dram_tensor("w", [128, KT, OT, 128], F32R, kind="ExternalInput")
    thr_d = nc.dram_tensor("thr", [128, OT, BL], F32, kind="ExternalInput")
    u0_d = nc.dram_tensor("u0", [128, OT, BL], F32, kind="ExternalInput")
    out_d = nc.dram_tensor("out", [128, OT, T, BL], F32, kind="ExternalOutput")

    with tile.TileContext(nc) as tc:
        with (
            tc.tile_pool(name="wpool", bufs=1) as wpool,
            tc.tile_pool(name="xpool", bufs=36) as xpool,
            tc.tile_pool(name="state", bufs=1) as state,
            tc.tile_pool(name="spkpool", bufs=2) as spkpool,
            tc.tile_pool(name="psum", bufs=2, space="PSUM") as psum_pool,
        ):
            # Weights: split DMA by kt-quarters so the first matmuls can
            # start before the full 4 MiB lands (gpsimd queue, parallel
            # to the x-prefetch on the sync queue).
            w_all = wpool.tile([128, KT, OT, 128], F32R)
            for q in range(4):
                nc.gpsimd.dma_start(w_all[:, q * 4:(q + 1) * 4],
                                    w_d[:, q * 4:(q + 1) * 4])

            # Persistent state tiles
            u = state.tile([128, OT, BL], F32)
            thr_t = state.tile([128, OT, BL], F32)
            nc.gpsimd.dma_start(u[:], u0_d[:])
            nc.gpsimd.dma_start(thr_t[:], thr_d[:])

            for bi in range(NBLK):
                # x tiles for this block (prefetch paced by pool depth)
                xts = []
                for kt in range(KT):
                    xt = xpool.tile([128, BLK], F32R, tag="xt",
                                    name=f"xt_{bi}_{kt}")
                    nc.sync.dma_start(
                        xt[:], xT_d[kt * 128:(kt + 1) * 128,
                                    bi * BLK:(bi + 1) * BLK])
                    xts.append(xt)

                ps = psum_pool.tile([128, OT, BLK], F32, tag="ps",
                                    name=f"ps_{bi}")
                for ot in range(OT):
                    for kt in range(KT):
                        nc.tensor.matmul(
                            ps[:, ot, :],
                            w_all[:, kt, ot, :],
                            xts[kt][:],
                            start=(kt == 0),
                            stop=(kt == KT - 1),
                        )

                # LIF steps consuming this block's PSUM
                spkb = spkpool.tile([128, OT, TBLK, BL], F32, tag="spk",
                                    name=f"spk_{bi}")
                for tl in range(TBLK):
                    nc.vector.tensor_scalar_mul(u[:], u[:], DECAY)
                    nc.vector.tensor_tensor(
                        u[:], u[:], ps[:, :, tl * BL:(tl + 1) * BL],
                        op=ALU.add)
                    nc.vector.tensor_tensor(
                        spkb[:, :, tl, :], u[:], thr_t[:], op=ALU.is_gt)
                    nc.vector.tensor_tensor(
                        u[:], u[:], spkb[:, :, tl, :], op=ALU.subtract)
                nc.gpsimd.dma_start(
                    out_d[:, :, bi * TBLK:(bi + 1) * TBLK, :], spkb[:])

    nc.compile()
    return nc


def _get_nc():
    if "nc" not in _cache:
        _cache["nc"] = _build_nc()
    return _cache["nc"]


def kernel(x_seq: np.ndarray, W: np.ndarray, b: np.ndarray) -> np.ndarray:
    nc = _get_nc()

    # Two distinct x shards (one per batch half), shared by 4 cores each.
    xTs = []
    for h in range(GB):
        xs = np.ascontiguousarray(
            x_seq[:, h * BL:(h + 1) * BL, :], dtype=np.float32)
        xTs.append(np.ascontiguousarray(xs.reshape(T * BL, I).T))

    in_maps = []
    for c in range(N_CORES):
        g, h = c // GB, c % GB
        w_c = W[g * OL:(g + 1) * OL, :].astype(np.float32)      # [OL, I]
        wTc = np.ascontiguousarray(w_c.T)                       # [I, OL]
        wp = np.ascontiguousarray(
            wTc.reshape(KT, 128, OT, 128).transpose(1, 0, 2, 3))
        b_c = b[g * OL:(g + 1) * OL].astype(np.float32)         # [OL]
        shift = b_c / (1.0 - DECAY)
        thr = (THR - shift).reshape(OT, 128).T                  # [128, OT]
        u0 = (-shift).reshape(OT, 128).T
        thr_tile = np.ascontiguousarray(
            np.broadcast_to(thr[:, :, None], (128, OT, BL)), dtype=np.float32)
        u0_tile = np.ascontiguousarray(
            np.broadcast_to(u0[:, :, None], (128, OT, BL)), dtype=np.float32)
        in_maps.append({
            "xT": xTs[h], "w": wp, "thr": thr_tile, "u0": u0_tile,
        })

    res = bass_utils.run_bass_kernel_spmd(nc, in_maps, core_ids=list(range(N_CORES)))
    global LAST_RESULT
    LAST_RESULT = res

    # Assemble: out_c[op, ot, t, b] -> [t, b, ot*128+op] per core block
    out = np.empty((T, B, O), dtype=np.float32)
    for c in range(N_CORES):
        g, h = c // GB, c % GB
        oc = res.results[c]["out"]  # [128, OT, T, BL]
        out[:, h * BL:(h + 1) * BL, g * OL:(g + 1) * OL] = (
            oc.transpose(2, 3, 1, 0).reshape(T, BL, OL))
    return out


LAST_RESULT = None


# revision 8
# speedup vs baseline: 2.1666x; 1.0803x over previous
"""LIF layer (T=64, B=128, 2048->2048) on 8 trn2 NeuronCores.

Sharding: 4-way over out_dim x 2-way over batch. Core (g, h) owns
out channels [g*512, (g+1)*512) and batch rows [h*64, (h+1)*64).

Per core:
  GEMM  cur[o, (t,b)] = sum_i W[o,i] * x[t,b,i] as a single f32r pass
        (fp22-ish precision, 1 cyc/row, measured ~0.015 rel on spikes)
  SCAN  64 sequential LIF steps on [128, 4, 64] state tiles (DVE),
        reading cur straight out of PSUM.
Bias is folded away via the change of variable u = mem - b/(1-decay),
turning the per-step bias add into a per-channel spike threshold
(THR=1 so the reset subtract is just u -= spk).

Blocks of 512 columns (8 timesteps): PSUM tile [128, 4ot, 512] = 4
banks, double-buffered. Matmuls run ot-outer/kt-inner so each bank
sees a 16-matmul accumulation run (avoids bank-cycling micro-idles).

Host-side prep: transpose x to [I, T*64] per batch half, slice/pack W,
precompute threshold/init tiles; final output is a transpose+concat.
"""

import math

import numpy as np

import concourse.bacc as bacc
import concourse.mybir as mybir
import concourse.tile as tile
from concourse import bass_utils

# Problem constants (hardcoded per contract)
T, B, I, O = 64, 128, 2048, 2048
N_CORES = 8
GO, GB = 4, 2              # out-groups x batch-groups
OL = O // GO               # 512 out-channels per core
OT = OL // 128             # 4 out tiles
BL = B // GB               # 64 batch rows per core
COLS = T * BL              # 4096 (t,b) columns per core
KT = I // 128              # 16 k-tiles
NBLK = 8                   # col-blocks per core
BLK = COLS // NBLK         # 512 cols = 8 timesteps per block
TBLK = BLK // BL           # 8 timesteps per block
TAU, THR = 2.0, 1.0
DECAY = math.exp(-1.0 / TAU)

F32 = mybir.dt.float32
F32R = mybir.dt.float32r
ALU = mybir.AluOpType

MODE = "f32r-tp4dp2"

_cache = {}


def _build_nc():
    nc = bacc.Bacc(trn_type="TRN2", target_bir_lowering=False)

    xT_d = nc.dram_tensor("xT", [128, KT, COLS], F32R, kind="ExternalInput")
    w_d = nc.dram_tensor("w", [128, KT, OT, 128], F32R, kind="ExternalInput")
    thr_d = nc.dram_tensor("thr", [128, OT, BL], F32, kind="ExternalInput")
    u0_d = nc.dram_tensor("u0", [128, OT, BL], F32, kind="ExternalInput")
    out_d = nc.dram_tensor("out", [128, OT, T, BL], F32, kind="ExternalOutput")

    with tile.TileContext(nc) as tc:
        with (
            tc.tile_pool(name="wpool", bufs=1) as wpool,
            tc.tile_pool(name="xpool", bufs=6) as xpool,
            tc.tile_pool(name="state", bufs=1) as state,
            tc.tile_pool(name="spkpool", bufs=2) as spkpool,
            tc.tile_pool(name="psum", bufs=2, space="PSUM") as psum_pool,
        ):
            # Weights: split DMA by kt-quarters so the first matmuls can
            # start before the full 4 MiB lands (gpsimd queue, parallel
            # to the x-prefetch on the sync queue).
            w_all = wpool.tile([128, KT, OT, 128], F32R)
            for q in range(4):
                nc.gpsimd.dma_start(w_all[:, q * 4:(q + 1) * 4],
                                    w_d[:, q * 4:(q + 1) * 4])

            # Persistent state tiles
            u = state.tile([128, OT, BL], F32)
            thr_t = state.tile([128, OT, BL], F32)
            nc.gpsimd.dma_start(u[:], u0_d[:])
            nc.gpsimd.dma_start(thr_t[:], thr_d[:])

            for bi in range(NBLK):
                # x for this block: two half-block DMAs (kt 0-7, 8-15)
                xts = []
                for half in range(2):
                    xt = xpool.tile([128, KT // 2, BLK], F32R, tag="xt",
                                    name=f"xt_{bi}_{half}")
                    nc.sync.dma_start(
                        xt[:], xT_d[:, half * 8:(half + 1) * 8,
                                    bi * BLK:(bi + 1) * BLK])
                    xts.append(xt)

                ps = psum_pool.tile([128, OT, BLK], F32, tag="ps",
                                    name=f"ps_{bi}")
                for ot in range(OT):
                    for kt in range(KT):
                        nc.tensor.matmul(
                            ps[:, ot, :],
                            w_all[:, kt, ot, :],
                            xts[kt // 8][:, kt % 8, :],
                            start=(kt == 0),
                            stop=(kt == KT - 1),
                        )

                # LIF steps consuming this block's PSUM
                spkb = spkpool.tile([128, OT, TBLK, BL], F32, tag="spk",
                                    name=f"spk_{bi}")
                for tl in range(TBLK):
                    nc.vector.scalar_tensor_tensor(
                        u[:], u[:], DECAY, ps[:, :, tl * BL:(tl + 1) * BL],
                        op0=ALU.mult, op1=ALU.add)
                    nc.vector.tensor_tensor(
                        spkb[:, :, tl, :], u[:], thr_t[:], op=ALU.is_gt)
                    nc.vector.tensor_tensor(
                        u[:], u[:], spkb[:, :, tl, :], op=ALU.subtract)
                nc.gpsimd.dma_start(
                    out_d[:, :, bi * TBLK:(bi + 1) * TBLK, :], spkb[:])

    nc.compile()
    return nc


def _get_nc():
    if "nc" not in _cache:
        _cache["nc"] = _build_nc()
    return _cache["nc"]


def kernel(x_seq: np.ndarray, W: np.ndarray, b: np.ndarray) -> np.ndarray:
    nc = _get_nc()

    # Two distinct x shards (one per batch half), shared by 4 cores each.
    # Packed as [128(p), KT, COLS] so block DMAs are single 3D transfers.
    xTs = []
    for h in range(GB):
        xs = np.ascontiguousarray(
            x_seq[:, h * BL:(h + 1) * BL, :], dtype=np.float32)
        xT = xs.reshape(T * BL, I).T  # [I, COLS]
        xTs.append(np.ascontiguousarray(
            xT.reshape(KT, 128, COLS).transpose(1, 0, 2)))

    in_maps = []
    for c in range(N_CORES):
        g, h = c // GB, c % GB
        w_c = W[g * OL:(g + 1) * OL, :].astype(np.float32)      # [OL, I]
        wTc = np.ascontiguousarray(w_c.T)                       # [I, OL]
        wp = np.ascontiguousarray(
            wTc.reshape(KT, 128, OT, 128).transpose(1, 0, 2, 3))
        b_c = b[g * OL:(g + 1) * OL].astype(np.float32)         # [OL]
        shift = b_c / (1.0 - DECAY)
        thr = (THR - shift).reshape(OT, 128).T                  # [128, OT]
        u0 = (-shift).reshape(OT, 128).T
        thr_tile = np.ascontiguousarray(
            np.broadcast_to(thr[:, :, None], (128, OT, BL)), dtype=np.float32)
        u0_tile = np.ascontiguousarray(
            np.broadcast_to(u0[:, :, None], (128, OT, BL)), dtype=np.float32)
        in_maps.append({
            "xT": xTs[h], "w": wp, "thr": thr_tile, "u0": u0_tile,
        })

    res = bass_utils.run_bass_kernel_spmd(nc, in_maps, core_ids=list(range(N_CORES)))
    global LAST_RESULT
    LAST_RESULT = res

    # Assemble: out_c[op, ot, t, b] -> [t, b, ot*128+op] per core block
    out = np.empty((T, B, O), dtype=np.float32)
    for c in range(N_CORES):
        g, h = c // GB, c % GB
        oc = res.results[c]["out"]  # [128, OT, T, BL]
        out[:, h * BL:(h + 1) * BL, g * OL:(g + 1) * OL] = (
            oc.transpose(2, 3, 1, 0).reshape(T, BL, OL))
    return out


LAST_RESULT = None


# revision 14
# speedup vs baseline: 2.2235x; 1.0262x over previous
"""LIF layer (T=64, B=128, 2048->2048) on 8 trn2 NeuronCores.

Sharding: 4-way over out_dim x 2-way over batch. Core (g, h) owns
out channels [g*512, (g+1)*512) and batch rows [h*64, (h+1)*64).

Per core:
  GEMM  cur[o, (t,b)] = sum_i W[o,i] * x[t,b,i] as a single f32r pass
        (fp22-ish precision, 1 cyc/row, measured ~0.015 rel on spikes)
  SCAN  64 sequential LIF steps on [128, 4, 64] state tiles (DVE),
        reading cur straight out of PSUM.
Bias is folded away via the change of variable u = mem - b/(1-decay),
turning the per-step bias add into a per-channel spike threshold
(THR=1 so the reset subtract is just u -= spk).

Blocks of 512 columns (8 timesteps): PSUM tile [128, 4ot, 512] = 4
banks, double-buffered. Matmuls run ot-outer/kt-inner so each bank
sees a 16-matmul accumulation run (avoids bank-cycling micro-idles).

Host-side prep: transpose x to [I, T*64] per batch half, slice/pack W,
precompute threshold/init tiles; final output is a transpose+concat.
"""

import math

import numpy as np

import concourse.bacc as bacc
import concourse.mybir as mybir
import concourse.tile as tile
from concourse import bass_utils

# Problem constants (hardcoded per contract)
T, B, I, O = 64, 128, 2048, 2048
N_CORES = 8
GO, GB = 4, 2              # out-groups x batch-groups
OL = O // GO               # 512 out-channels per core
OT = OL // 128             # 4 out tiles
BL = B // GB               # 64 batch rows per core
COLS = T * BL              # 4096 (t,b) columns per core
KT = I // 128              # 16 k-tiles
NBLK = 8                   # col-blocks per core
BLK = COLS // NBLK         # 512 cols = 8 timesteps per block
TBLK = BLK // BL           # 8 timesteps per block
TAU, THR = 2.0, 1.0
DECAY = math.exp(-1.0 / TAU)

F32 = mybir.dt.float32
F32R = mybir.dt.float32r
ALU = mybir.AluOpType

MODE = "f32r-tp4dp2"

_cache = {}


def _build_nc():
    nc = bacc.Bacc(trn_type="TRN2", target_bir_lowering=False)

    xT_d = nc.dram_tensor("xT", [128, KT, COLS], F32R, kind="ExternalInput")
    w_d = nc.dram_tensor("w", [128, KT, OT, 128], F32R, kind="ExternalInput")
    thr_d = nc.dram_tensor("thr", [128, OT, BL], F32, kind="ExternalInput")
    u0_d = nc.dram_tensor("u0", [128, OT, BL], F32, kind="ExternalInput")
    out_d = nc.dram_tensor("out", [128, T, OT, BL], F32, kind="ExternalOutput")

    with tile.TileContext(nc) as tc:
        with (
            tc.tile_pool(name="wpool", bufs=1) as wpool,
            tc.tile_pool(name="xpool", bufs=7) as xpool,
            tc.tile_pool(name="state", bufs=1) as state,
            tc.tile_pool(name="spkpool", bufs=4) as spkpool,
            tc.tile_pool(name="psum", bufs=2, space="PSUM") as psum_pool,
        ):
            # Persistent state tiles (tiny, load first)
            u = state.tile([128, OT, BL], F32)
            thr_t = state.tile([128, OT, BL], F32)
            nc.gpsimd.dma_start(u[:], u0_d[:])
            nc.gpsimd.dma_start(thr_t[:], thr_d[:])

            # Weights as separate chunk tiles so matmuls gate only on the
            # chunk they read (tile-granular deps): kt ranges [0,2,4,8,16).
            w_bounds = [0, 2, 4, 8, KT]
            w_chunks = []
            for q in range(4):
                lo, hi = w_bounds[q], w_bounds[q + 1]
                wc = wpool.tile([128, hi - lo, OT, 128], F32R,
                                name=f"w_{q}")
                nc.gpsimd.dma_start(wc[:], w_d[:, lo:hi])
                w_chunks.append((lo, wc))

            def w_tile(kt):
                for lo, wc in reversed(w_chunks):
                    if kt >= lo:
                        return wc[:, kt - lo]
                raise AssertionError

            for bi in range(NBLK):
                # x for this block: kt-chunked DMAs (block 0 staircased so
                # the first matmul can start ASAP)
                x_bounds = [0, 2, 4, 8, KT] if bi == 0 else [0, 8, KT]
                xts = []
                for xi in range(len(x_bounds) - 1):
                    lo, hi = x_bounds[xi], x_bounds[xi + 1]
                    xt = xpool.tile([128, KT // 2, BLK], F32R, tag="xt",
                                    name=f"xt_{bi}_{xi}")
                    nc.sync.dma_start(
                        xt[:, :hi - lo],
                        xT_d[:, lo:hi, bi * BLK:(bi + 1) * BLK])
                    xts.append((lo, hi, xt))

                def x_slice(kt):
                    for lo, hi, xt in xts:
                        if lo <= kt < hi:
                            return xt[:, kt - lo, :]
                    raise AssertionError

                ps = psum_pool.tile([128, OT, BLK], F32, tag="ps",
                                    name=f"ps_{bi}")
                for ot in range(OT):
                    for kt in range(KT):
                        nc.tensor.matmul(
                            ps[:, ot, :],
                            w_tile(kt)[:, ot, :],
                            x_slice(kt),
                            start=(kt == 0),
                            stop=(kt == KT - 1),
                        )

                # LIF steps consuming this block's PSUM; spikes land in two
                # half-block buffers so the out-DMA overlaps the scan.
                for hf in range(2):
                    spkb = spkpool.tile([128, TBLK // 2, OT, BL], F32,
                                        tag="spk", name=f"spk_{bi}_{hf}")
                    for tj in range(TBLK // 2):
                        tl = hf * (TBLK // 2) + tj
                        nc.vector.scalar_tensor_tensor(
                            u[:], u[:], DECAY,
                            ps[:, :, tl * BL:(tl + 1) * BL],
                            op0=ALU.mult, op1=ALU.add)
                        nc.vector.tensor_tensor(
                            spkb[:, tj], u[:], thr_t[:], op=ALU.is_gt)
                        nc.vector.tensor_tensor(
                            u[:], u[:], spkb[:, tj], op=ALU.subtract)
                    t0 = bi * TBLK + hf * (TBLK // 2)
                    nc.gpsimd.dma_start(
                        out_d[:, t0:t0 + TBLK // 2], spkb[:])

    nc.compile()
    return nc


def _get_nc():
    if "nc" not in _cache:
        _cache["nc"] = _build_nc()
    return _cache["nc"]


def kernel(x_seq: np.ndarray, W: np.ndarray, b: np.ndarray) -> np.ndarray:
    nc = _get_nc()

    # Two distinct x shards (one per batch half), shared by 4 cores each.
    # Packed as [128(p), KT, COLS] so block DMAs are single 3D transfers.
    xTs = []
    for h in range(GB):
        xs = np.ascontiguousarray(
            x_seq[:, h * BL:(h + 1) * BL, :], dtype=np.float32)
        xT = xs.reshape(T * BL, I).T  # [I, COLS]
        xTs.append(np.ascontiguousarray(
            xT.reshape(KT, 128, COLS).transpose(1, 0, 2)))

    in_maps = []
    for c in range(N_CORES):
        g, h = c // GB, c % GB
        w_c = W[g * OL:(g + 1) * OL, :].astype(np.float32)      # [OL, I]
        wTc = np.ascontiguousarray(w_c.T)                       # [I, OL]
        wp = np.ascontiguousarray(
            wTc.reshape(KT, 128, OT, 128).transpose(1, 0, 2, 3))
        b_c = b[g * OL:(g + 1) * OL].astype(np.float32)         # [OL]
        shift = b_c / (1.0 - DECAY)
        thr = (THR - shift).reshape(OT, 128).T                  # [128, OT]
        u0 = (-shift).reshape(OT, 128).T
        thr_tile = np.ascontiguousarray(
            np.broadcast_to(thr[:, :, None], (128, OT, BL)), dtype=np.float32)
        u0_tile = np.ascontiguousarray(
            np.broadcast_to(u0[:, :, None], (128, OT, BL)), dtype=np.float32)
        in_maps.append({
            "xT": xTs[h], "w": wp, "thr": thr_tile, "u0": u0_tile,
        })

    res = bass_utils.run_bass_kernel_spmd(nc, in_maps, core_ids=list(range(N_CORES)))
    global LAST_RESULT
    LAST_RESULT = res

    # Assemble: out_c[op, ot, t, b] -> [t, b, ot*128+op] per core block
    out = np.empty((T, B, O), dtype=np.float32)
    for c in range(N_CORES):
        g, h = c // GB, c % GB
        oc = res.results[c]["out"]  # [128, T, OT, BL]
        out[:, h * BL:(h + 1) * BL, g * OL:(g + 1) * OL] = (
            oc.transpose(1, 3, 2, 0).reshape(T, BL, OL))
    return out


LAST_RESULT = None
